# revision 14
# baseline (speedup 1.0000x reference)
"""Trainium2 Bass kernel for DifferentiableDefocusRenderer.

Math (mirrors the reference):
  planes = linspace(0, 50, 32); per-plane depthwise Gaussian blur of
  sharp_image (separable, kernel k<=31, truncated+renormalized), output =
  per-pixel hard select of the blurred plane by CoC bucket.

Distribution: pure data parallel, 8 cores = (batch b in 0..3) x (H half).
Each core computes [3, 256, 512] of output for its (b, half).

Per-core pipeline (all-plane dense, PE-heavy):
  pass A (column conv, all 32 planes in one matmul stream):
      C[x, y, i] = sum_k X[k, x] * T1[k, (y,i)]   (role-swapped matmul:
      stationary = X y-window tile, moving = multi-plane Toeplitz T1)
  pass B (row conv, per plane, 3-window PSUM accumulation):
      Q_i[xo, y] = sum_k T2_i[k, xo] * C[k, y, i]
  select: copy_predicated(acc_lane, mask_i, Q_i) with one-hot plane masks,
      4 planes per PSUM group, 4 disjoint lanes folded at the end,
  then PE-transpose acc back to [y, x] and DMA out.
"""

import os
import sys

import numpy as np
import ml_dtypes

sys.path.insert(0, "/opt/trn_rl_repo")

B, C, H, W = 4, 3, 512, 512
MAX_COC = 50.0
NPLANES = 32
HALF = 256          # output rows per core
YT = 64             # output rows per pass-A y-tile
NT = HALF // YT     # 4 y-tiles
NS = W // 128       # 4 x slices
NCHUNK = 4          # pass-A moving chunks: 64 q x 32 i = 2048 = 4 x 512
BF16 = ml_dtypes.bfloat16

_CACHE = {}


# ----------------------------------------------------------------------------
# host-side tables (exactly mirroring reference kernel construction)
# ----------------------------------------------------------------------------

def _gaussian_kernel_1d(coc_value):
    # mirrors reference._gaussian_kernel_np (1-D factor of the outer product)
    sigma = coc_value / 2.355
    k = int(2 * coc_value + 1)
    if k % 2 == 0:
        k += 1
    k = min(k, 31)
    coords = np.arange(k, dtype=np.float32) - (k // 2)
    g = np.exp(-coords ** 2 / (2.0 * sigma ** 2))
    g = g / g.sum()
    return g.astype(np.float32)  # [k]


def _plane_kernels():
    """g31[i] in R^31, centered; plane 0 = identity delta."""
    planes = np.linspace(0.0, MAX_COC, NPLANES, dtype=np.float32)
    g31 = np.zeros((NPLANES, 31), dtype=np.float32)
    for i in range(NPLANES):
        coc = float(planes[i])
        if coc < 0.5:
            g31[i, 15] = 1.0
        else:
            g = _gaussian_kernel_1d(coc)
            k = g.shape[0]
            off = (31 - k) // 2
            g31[i, off:off + k] = g
    return planes, g31


def _host_tables():
    planes, g31 = _plane_kernels()

    # T1[k, c, 32*qq + i] = g31[i][k - (16c+qq) - 17]
    t1 = np.zeros((128, NCHUNK, 512), dtype=np.float32)
    for c in range(NCHUNK):
        for qq in range(16):
            q = 16 * c + qq
            for k in range(128):
                idx = k - q - 17
                if 0 <= idx <= 30:
                    t1[k, c, 32 * qq:32 * qq + 32] = g31[:, idx]

    # T2 main [k, i, m] = g31[i][k - m + 15]
    t2m = np.zeros((128, NPLANES, 128), dtype=np.float32)
    for k in range(128):
        lo = max(0, k - 15)
        hi = min(128, k + 16)
        for m in range(lo, hi):
            t2m[k, :, m] = g31[:, k - m + 15]
    # T2 left: rows kk = k_loc-64 of the window one tile to the left:
    #   [kk, i, m] = g31[i][kk - m - 49]   (nonzero kk-m in [49,79))
    # stored padded to 128 partitions; rows [64:128) hold the window rows
    # kk2 = k_loc - 64: g-idx = (kk2-64) - m - 49... see below
    t2l = np.zeros((128, NPLANES, 128), dtype=np.float32)
    for kk in range(64, 128):
        for m in range(128):
            idx = (kk - 64) - m - 49
            if 0 <= idx <= 30:
                t2l[kk, :, m] = g31[:, idx]
    # T2 right: rows k_loc in [0,32) of the window one tile to the right:
    #   [kk, i, m] = g31[i][kk - m + 143]  (nonzero m-kk in [113,143))
    t2r = np.zeros((32, NPLANES, 128), dtype=np.float32)
    for kk in range(32):
        for m in range(128):
            idx = kk - m + 143
            if 0 <= idx <= 30:
                t2r[kk, :, m] = g31[:, idx]

    ident = np.eye(128, dtype=np.float32)
    return (planes, t1.astype(BF16), t2m.astype(BF16), t2l.astype(BF16),
            t2r.astype(BF16), ident)


def _plane_index(coc):
    """Exact bucket index per pixel, replicating reference fp32 comparisons."""
    planes = np.linspace(0.0, MAX_COC, NPLANES, dtype=np.float32)
    bnd = ((planes[:-1] + planes[1:]) / np.float32(2.0)).astype(np.float32)
    coc = coc.astype(np.float32)
    p = np.zeros(coc.shape, dtype=np.int32)
    for i in range(NPLANES - 1):
        p += (coc > bnd[i]).astype(np.int32)
    return p  # [H, W] int in [0, 31]


# ----------------------------------------------------------------------------
# device program
# ----------------------------------------------------------------------------

def _build_program():
    import concourse.bass as bass
    import concourse.bacc as bacc
    import concourse.mybir as mybir
    import concourse.tile as tile
    from concourse._compat import axon_active

    dt = mybir.dt
    nc = bacc.Bacc("TRN2", target_bir_lowering=False,
                   debug=False, enable_asserts=False, num_devices=8)

    xin_d = nc.dram_tensor("xin", [C, NT, 128, 512], dt.bfloat16,
                           kind="ExternalInput")
    t1_d = nc.dram_tensor("t1", [128, NCHUNK, 512], dt.bfloat16,
                          kind="ExternalInput")
    t2m_d = nc.dram_tensor("t2m", [128, NPLANES, 128], dt.bfloat16,
                           kind="ExternalInput")
    t2l_d = nc.dram_tensor("t2l", [128, NPLANES, 128], dt.bfloat16,
                           kind="ExternalInput")
    t2r_d = nc.dram_tensor("t2r", [32, NPLANES, 128], dt.bfloat16,
                           kind="ExternalInput")
    pmap_d = nc.dram_tensor("pmap", [128, NS, HALF], dt.bfloat16,
                            kind="ExternalInput")
    id_d = nc.dram_tensor("ident", [128, 128], dt.float32,
                          kind="ExternalInput")
    out_d = nc.dram_tensor("out", [C, 2, 128, 512], dt.float32,
                           kind="ExternalOutput")

    with tile.TileContext(nc) as tc:
        with (
            tc.tile_pool(name="const", bufs=1) as const_pool,
            tc.tile_pool(name="cbuf", bufs=1) as c_pool,
            tc.tile_pool(name="xin", bufs=3) as x_pool,
            tc.tile_pool(name="work", bufs=2) as w_pool,
            tc.tile_pool(name="accp", bufs=1) as acc_pool,
            tc.tile_pool(name="psA", bufs=2, space="PSUM") as psA,
            tc.tile_pool(name="psB", bufs=2, space="PSUM") as psB,
            tc.tile_pool(name="psT", bufs=2, space="PSUM") as psT,
        ):
            # ---- constants ----
            t1_s = const_pool.tile([128, NCHUNK, 512], dt.bfloat16, tag="t1")
            nc.sync.dma_start(t1_s[:], t1_d.ap()[:])
            t2m_s = const_pool.tile([128, NPLANES, 128], dt.bfloat16, tag="t2m")
            nc.sync.dma_start(t2m_s[:], t2m_d.ap()[:])
            t2l_s = const_pool.tile([128, NPLANES, 128], dt.bfloat16, tag="t2l")
            nc.sync.dma_start(t2l_s[:], t2l_d.ap()[:])
            t2r_s = const_pool.tile([32, NPLANES, 128], dt.bfloat16, tag="t2r")
            nc.sync.dma_start(t2r_s[:], t2r_d.ap()[:])
            pmap_s = const_pool.tile([128, NS, HALF], dt.bfloat16, tag="pmap")
            nc.sync.dma_start(pmap_s[:], pmap_d.ap()[:])
            id_s = const_pool.tile([128, 128], dt.float32, tag="ident")
            nc.sync.dma_start(id_s[:], id_d.ap()[:])

            # ---- one-hot plane masks (shared across channels) ----
            # masks[i] : [128, NS, HALF] uint8 over all x-slices
            masks = {}
            for i in range(NPLANES):
                mk = const_pool.tile([128, NS, HALF], dt.uint8,
                                     tag=f"mask{i}", name=f"mask{i}")
                nc.vector.tensor_scalar(
                    mk[:], pmap_s[:],
                    float(i), None,
                    mybir.AluOpType.is_equal)
                masks[i] = mk

            for ch in range(C):
                # ---- pass A: column conv, C[x, y, i] ----
                c_sb = [c_pool.tile([128, NPLANES, HALF], dt.bfloat16,
                                    tag=f"c{s}", name=f"c{s}") for s in range(NS)]
                for t in range(NT):
                    xt = x_pool.tile([128, 512], dt.bfloat16, tag="xt")
                    nc.sync.dma_start(xt[:], xin_d.ap()[ch, t])
                    for s in range(NS):
                        for c in range(NCHUNK):
                            pa = psA.tile([128, 512], dt.float32, tag="pa")
                            nc.tensor.matmul(
                                pa[:], xt[:, 128 * s:128 * (s + 1)],
                                t1_s[:, c, :], start=True, stop=True)
                            # psum [x, (qq,i)] -> C[x, i, y=64t+16c+qq]
                            y0 = YT * t + 16 * c
                            if (s + c) % 2 == 0:
                                nc.scalar.copy(
                                    c_sb[s][:, :, y0:y0 + 16],
                                    pa.rearrange("p (qq i) -> p i qq", qq=16))
                            else:
                                nc.vector.tensor_copy(
                                    c_sb[s][:, :, y0:y0 + 16],
                                    pa.rearrange("p (qq i) -> p i qq", qq=16))

                # ---- pass B + select ----
                # accL[j] accumulates planes i = 4g+j over all s
                accL = [acc_pool.tile([128, NS, HALF], dt.float32,
                                      tag=f"accl{j}", name=f"accl{j}")
                        for j in range(4)]
                for j in range(4):
                    nc.gpsimd.memset(accL[j][:], 0.0)
                for g in range(8):
                    for j in range(4):
                        i = 4 * g + j
                        pb = psB.tile([128, NS, HALF], dt.float32, tag="pb",
                                      name="pb")
                        # bank A = regions s=0,1; bank B = s=2,3.
                        # order MMs to alternate banks so drains overlap;
                        # start=True = first MM per bank, stop=True = last.
                        seq = [("m", 0, True, False), ("m", 2, True, False),
                               ("m", 1, False, False), ("m", 3, False, False),
                               ("l", 1, False, False), ("r", 2, False, False),
                               ("r", 0, False, False), ("l", 2, False, False),
                               ("r", 1, False, True), ("l", 3, False, True)]
                        for kind, s, st, sp in seq:
                            if kind == "m":
                                nc.tensor.matmul(
                                    pb[:, s, :], t2m_s[:, i, :],
                                    c_sb[s][:, i, :], start=st, stop=sp)
                            elif kind == "l":
                                nc.tensor.matmul(
                                    pb[:, s, :], t2l_s[64:128, i, :],
                                    c_sb[s - 1][64:128, i, :],
                                    start=st, stop=sp)
                            else:
                                nc.tensor.matmul(
                                    pb[:, s, :], t2r_s[:, i, :],
                                    c_sb[s + 1][0:32, i, :],
                                    start=st, stop=sp)
                        nc.vector.copy_predicated(
                            accL[j][:], masks[i][:], pb[:])

                # ---- fold lanes, transpose to [y, x], store ----
                for u in range(2):
                    onat = w_pool.tile([128, 512], dt.float32, tag=f"onat{u}",
                                       name=f"onat{u}")
                    for s in range(NS):
                        accF = w_pool.tile([128, HALF], dt.float32, tag="accf",
                                           name="accf")
                        nc.gpsimd.tensor_add(accF[:], accL[0][:, s, :],
                                             accL[1][:, s, :])
                        nc.gpsimd.tensor_add(accF[:], accF[:],
                                             accL[2][:, s, :])
                        nc.gpsimd.tensor_add(accF[:], accF[:],
                                             accL[3][:, s, :])
                        tp = psT.tile([128, 128], dt.float32, tag="tp")
                        nc.tensor.transpose(
                            tp[:], accF[:, 128 * u:128 * (u + 1)], id_s[:])
                        nc.scalar.copy(onat[:, 128 * s:128 * (s + 1)], tp[:])
                    nc.sync.dma_start(out_d.ap()[ch, u], onat[:])

    nc.compile()
    return nc


# ----------------------------------------------------------------------------
# host orchestration
# ----------------------------------------------------------------------------

def _prepare_in_maps(sharp_image, coc_map):
    planes, t1, t2m, t2l, t2r, ident = _CACHE["tables"]
    p_full = {}
    in_maps = []
    for core in range(8):
        b, h = divmod(core, 2)
        y0 = HALF * h
        # X padded rows [-32, 288) local
        xpad = np.zeros((C, HALF + 64, W), dtype=np.float32)
        glo = y0 - 32
        ghi = y0 + HALF + 32
        clo, chi = max(0, glo), min(H, ghi)
        xpad[:, clo - glo:chi - glo, :] = sharp_image[b, :, clo:chi, :]
        xin = np.zeros((C, NT, 128, W), dtype=np.float32)
        for t in range(NT):
            xin[:, t] = xpad[:, YT * t:YT * t + 128, :]

        if b not in p_full:
            p_full[b] = _plane_index(coc_map[b, 0])
        p = p_full[b][y0:y0 + HALF, :]  # [HALF, W]
        # pmap[m, s, y] = p[y, 128s + m]
        pmap = np.ascontiguousarray(
            p.T.reshape(NS, 128, HALF).transpose(1, 0, 2)).astype(BF16)

        in_maps.append({
            "xin": xin.astype(BF16),
            "t1": t1, "t2m": t2m, "t2l": t2l, "t2r": t2r,
            "pmap": pmap, "ident": ident,
        })
    return in_maps


def _assemble(results):
    out = np.zeros((B, C, H, W), dtype=np.float32)
    for core in range(8):
        b, h = divmod(core, 2)
        r = results[core]["out"]  # [C, 2, 128, 512]
        out[b, :, HALF * h:HALF * (h + 1), :] = r.reshape(C, HALF, W)
    return out


def run(inputs, trace=False):
    from concourse import bass_utils
    if "tables" not in _CACHE:
        _CACHE["tables"] = _host_tables()
    if "nc" not in _CACHE:
        _CACHE["nc"] = _build_program()
    nc = _CACHE["nc"]
    in_maps = _prepare_in_maps(inputs["sharp_image"], inputs["coc_map"])
    res = bass_utils.run_bass_kernel_spmd(
        nc, in_maps, core_ids=list(range(8)), trace=trace)
    return _assemble(res.results), res


def kernel(**inputs):
    out, _ = run(inputs)
    return out


# revision 16
# speedup vs baseline: 1.4253x; 1.4253x over previous
"""Trainium2 Bass kernel for DifferentiableDefocusRenderer.

Math (mirrors the reference):
  planes = linspace(0, 50, 32); per-plane depthwise Gaussian blur of
  sharp_image (separable, kernel k<=31, truncated+renormalized), output =
  per-pixel hard select of the blurred plane by CoC bucket.

Distribution: pure data parallel, 8 cores = (batch b in 0..3) x (H half).
Each core computes [3, 256, 512] of output for its (b, half).

Per-core pipeline (all-plane dense, PE-heavy):
  pass A (column conv, all 32 planes in one matmul stream):
      C[x, y, i] = sum_k X[k, x] * T1[k, (y,i)]   (role-swapped matmul:
      stationary = X y-window tile, moving = multi-plane Toeplitz T1)
  pass B (row conv, per plane, 3-window PSUM accumulation):
      Q_i[xo, y] = sum_k T2_i[k, xo] * C[k, y, i]
  select: copy_predicated(acc_lane, mask_i, Q_i) with one-hot plane masks,
      4 planes per PSUM group, 4 disjoint lanes folded at the end,
  then PE-transpose acc back to [y, x] and DMA out.
"""

import os
import sys

import numpy as np
import ml_dtypes

sys.path.insert(0, "/opt/trn_rl_repo")

B, C, H, W = 4, 3, 512, 512
MAX_COC = 50.0
NPLANES = 32
HALF = 256          # output rows per core
YT = 64             # output rows per pass-A y-tile
NT = HALF // YT     # 4 y-tiles
NS = W // 128       # 4 x slices
NCHUNK = 4          # pass-A moving chunks: 64 q x 32 i = 2048 = 4 x 512
BF16 = ml_dtypes.bfloat16

_CACHE = {}


# ----------------------------------------------------------------------------
# host-side tables (exactly mirroring reference kernel construction)
# ----------------------------------------------------------------------------

def _gaussian_kernel_1d(coc_value):
    # mirrors reference._gaussian_kernel_np (1-D factor of the outer product)
    sigma = coc_value / 2.355
    k = int(2 * coc_value + 1)
    if k % 2 == 0:
        k += 1
    k = min(k, 31)
    coords = np.arange(k, dtype=np.float32) - (k // 2)
    g = np.exp(-coords ** 2 / (2.0 * sigma ** 2))
    g = g / g.sum()
    return g.astype(np.float32)  # [k]


def _plane_kernels():
    """g31[i] in R^31, centered; plane 0 = identity delta."""
    planes = np.linspace(0.0, MAX_COC, NPLANES, dtype=np.float32)
    g31 = np.zeros((NPLANES, 31), dtype=np.float32)
    for i in range(NPLANES):
        coc = float(planes[i])
        if coc < 0.5:
            g31[i, 15] = 1.0
        else:
            g = _gaussian_kernel_1d(coc)
            k = g.shape[0]
            off = (31 - k) // 2
            g31[i, off:off + k] = g
    return planes, g31


def _host_tables():
    planes, g31 = _plane_kernels()

    # T1[k, c, 32*qq + i] = g31[i][k - (16c+qq) - 17]
    t1 = np.zeros((128, NCHUNK, 512), dtype=np.float32)
    for c in range(NCHUNK):
        for qq in range(16):
            q = 16 * c + qq
            for k in range(128):
                idx = k - q - 17
                if 0 <= idx <= 30:
                    t1[k, c, 32 * qq:32 * qq + 32] = g31[:, idx]

    # T2 main [k, i, m] = g31[i][k - m + 15]
    t2m = np.zeros((128, NPLANES, 128), dtype=np.float32)
    for k in range(128):
        lo = max(0, k - 15)
        hi = min(128, k + 16)
        for m in range(lo, hi):
            t2m[k, :, m] = g31[:, k - m + 15]
    # T2 left: rows kk = k_loc-64 of the window one tile to the left:
    #   [kk, i, m] = g31[i][kk - m - 49]   (nonzero kk-m in [49,79))
    # stored padded to 128 partitions; rows [64:128) hold the window rows
    # kk2 = k_loc - 64: g-idx = (kk2-64) - m - 49... see below
    t2l = np.zeros((128, NPLANES, 128), dtype=np.float32)
    for kk in range(64, 128):
        for m in range(128):
            idx = (kk - 64) - m - 49
            if 0 <= idx <= 30:
                t2l[kk, :, m] = g31[:, idx]
    # T2 right: rows k_loc in [0,32) of the window one tile to the right:
    #   [kk, i, m] = g31[i][kk - m + 143]  (nonzero m-kk in [113,143))
    t2r = np.zeros((32, NPLANES, 128), dtype=np.float32)
    for kk in range(32):
        for m in range(128):
            idx = kk - m + 143
            if 0 <= idx <= 30:
                t2r[kk, :, m] = g31[:, idx]

    ident = np.eye(128, dtype=np.float32)
    return (planes, t1.astype(BF16), t2m.astype(BF16), t2l.astype(BF16),
            t2r.astype(BF16), ident)


def _plane_index(coc):
    """Exact bucket index per pixel, replicating reference fp32 comparisons."""
    planes = np.linspace(0.0, MAX_COC, NPLANES, dtype=np.float32)
    bnd = ((planes[:-1] + planes[1:]) / np.float32(2.0)).astype(np.float32)
    coc = coc.astype(np.float32)
    p = np.zeros(coc.shape, dtype=np.int32)
    for i in range(NPLANES - 1):
        p += (coc > bnd[i]).astype(np.int32)
    return p  # [H, W] int in [0, 31]


# ----------------------------------------------------------------------------
# device program
# ----------------------------------------------------------------------------

def _build_program():
    import concourse.bass as bass
    import concourse.bacc as bacc
    import concourse.mybir as mybir
    import concourse.tile as tile
    from concourse._compat import axon_active

    dt = mybir.dt
    nc = bacc.Bacc("TRN2", target_bir_lowering=False,
                   debug=False, enable_asserts=False, num_devices=8)

    xin_d = nc.dram_tensor("xin", [C, NT, 128, 512], dt.bfloat16,
                           kind="ExternalInput")
    t1_d = nc.dram_tensor("t1", [128, NCHUNK, 512], dt.bfloat16,
                          kind="ExternalInput")
    t2m_d = nc.dram_tensor("t2m", [128, NPLANES, 128], dt.bfloat16,
                           kind="ExternalInput")
    t2l_d = nc.dram_tensor("t2l", [128, NPLANES, 128], dt.bfloat16,
                           kind="ExternalInput")
    t2r_d = nc.dram_tensor("t2r", [32, NPLANES, 128], dt.bfloat16,
                           kind="ExternalInput")
    pmap_d = nc.dram_tensor("pmap", [128, NS, HALF], dt.bfloat16,
                            kind="ExternalInput")
    id_d = nc.dram_tensor("ident", [128, 128], dt.float32,
                          kind="ExternalInput")
    out_d = nc.dram_tensor("out", [C, 2, 128, 512], dt.float32,
                           kind="ExternalOutput")

    with tile.TileContext(nc) as tc:
        with (
            tc.tile_pool(name="const", bufs=1) as const_pool,
            tc.tile_pool(name="cbuf", bufs=1) as c_pool,
            tc.tile_pool(name="xin", bufs=3) as x_pool,
            tc.tile_pool(name="work", bufs=2) as w_pool,
            tc.tile_pool(name="accp", bufs=1) as acc_pool,
            tc.tile_pool(name="psA", bufs=3, space="PSUM") as psA,
            tc.tile_pool(name="psB", bufs=2, space="PSUM") as psB,
            tc.tile_pool(name="psT", bufs=1, space="PSUM") as psT,
        ):
            # ---- constants ----
            t1_s = const_pool.tile([128, NCHUNK, 512], dt.bfloat16, tag="t1")
            nc.sync.dma_start(t1_s[:], t1_d.ap()[:])
            t2m_s = const_pool.tile([128, NPLANES, 128], dt.bfloat16, tag="t2m")
            nc.sync.dma_start(t2m_s[:], t2m_d.ap()[:])
            t2l_s = const_pool.tile([128, NPLANES, 128], dt.bfloat16, tag="t2l")
            nc.sync.dma_start(t2l_s[:], t2l_d.ap()[:])
            t2r_s = const_pool.tile([32, NPLANES, 128], dt.bfloat16, tag="t2r")
            nc.sync.dma_start(t2r_s[:], t2r_d.ap()[:])
            pmap_s = const_pool.tile([128, NS, HALF], dt.bfloat16, tag="pmap")
            nc.sync.dma_start(pmap_s[:], pmap_d.ap()[:])
            id_s = const_pool.tile([128, 128], dt.float32, tag="ident")
            nc.sync.dma_start(id_s[:], id_d.ap()[:])

            masks = {}

            for ch in range(C):
                # ---- pass A: column conv, C[x, y, i] ----
                c_sb = [c_pool.tile([128, NPLANES, HALF], dt.bfloat16,
                                    tag=f"c{s}", name=f"c{s}") for s in range(NS)]
                for t in range(NT):
                    xt = x_pool.tile([128, 512], dt.bfloat16, tag="xt")
                    nc.sync.dma_start(xt[:], xin_d.ap()[ch, t])
                    for s in range(NS):
                        for c in range(NCHUNK):
                            pa = psA.tile([128, 512], dt.float32, tag="pa")
                            nc.tensor.matmul(
                                pa[:], xt[:, 128 * s:128 * (s + 1)],
                                t1_s[:, c, :], start=True, stop=True)
                            # psum [x, (qq,i)] -> C[x, i, y=64t+16c+qq]
                            y0 = YT * t + 16 * c
                            if (s + c) % 2 == 0:
                                nc.scalar.copy(
                                    c_sb[s][:, :, y0:y0 + 16],
                                    pa.rearrange("p (qq i) -> p i qq", qq=16))
                            else:
                                nc.vector.tensor_copy(
                                    c_sb[s][:, :, y0:y0 + 16],
                                    pa.rearrange("p (qq i) -> p i qq", qq=16))

                if ch == 0:
                    # one-hot plane masks (shared across channels); built
                    # here so they don't block the first pass A
                    for i in range(NPLANES):
                        mk = const_pool.tile([128, NS, HALF], dt.uint8,
                                             tag=f"mask{i}", name=f"mask{i}")
                        nc.vector.tensor_scalar(
                            mk[:], pmap_s[:],
                            float(i), None,
                            mybir.AluOpType.is_equal)
                        masks[i] = mk

                # ---- pass B + select ----
                acc = acc_pool.tile([128, NS, HALF], dt.float32,
                                    tag="acc", name="acc")
                nc.gpsimd.memset(acc[:], 0.0)
                for g in range(8):
                    for j in range(4):
                        i = 4 * g + j
                        pb = psB.tile([128, NS, HALF], dt.float32, tag="pb",
                                      name="pb")
                        # bank A = regions s=0,1; bank B = s=2,3.
                        # order MMs to alternate banks so drains overlap;
                        # start=True = first MM per bank, stop=True = last.
                        seq = [("m", 0, True, False), ("m", 1, False, False),
                               ("m", 2, True, False), ("m", 3, False, False),
                               ("l", 1, False, False), ("l", 2, False, False),
                               ("l", 3, False, False), ("r", 0, False, False),
                               ("r", 1, False, True), ("r", 2, False, True)]
                        for kind, s, st, sp in seq:
                            if kind == "m":
                                nc.tensor.matmul(
                                    pb[:, s, :], t2m_s[:, i, :],
                                    c_sb[s][:, i, :], start=st, stop=sp)
                            elif kind == "l":
                                nc.tensor.matmul(
                                    pb[:, s, :], t2l_s[64:128, i, :],
                                    c_sb[s - 1][64:128, i, :],
                                    start=st, stop=sp)
                            else:
                                nc.tensor.matmul(
                                    pb[:, s, :], t2r_s[:, i, :],
                                    c_sb[s + 1][0:32, i, :],
                                    start=st, stop=sp)
                        nc.vector.copy_predicated(
                            acc[:], masks[i][:], pb[:])

                # ---- fold lanes, transpose to [y, x], store ----
                for u in range(2):
                    onat = w_pool.tile([128, 512], dt.float32, tag=f"onat{u}",
                                       name=f"onat{u}")
                    for s in range(NS):
                        tp = psT.tile([128, 128], dt.float32, tag="tp")
                        nc.tensor.transpose(
                            tp[:], acc[:, s, 128 * u:128 * (u + 1)], id_s[:])
                        nc.scalar.copy(onat[:, 128 * s:128 * (s + 1)], tp[:])
                    nc.sync.dma_start(out_d.ap()[ch, u], onat[:])

    nc.compile()
    return nc


# ----------------------------------------------------------------------------
# host orchestration
# ----------------------------------------------------------------------------

def _prepare_in_maps(sharp_image, coc_map):
    planes, t1, t2m, t2l, t2r, ident = _CACHE["tables"]
    p_full = {}
    in_maps = []
    for core in range(8):
        b, h = divmod(core, 2)
        y0 = HALF * h
        # X padded rows [-32, 288) local
        xpad = np.zeros((C, HALF + 64, W), dtype=np.float32)
        glo = y0 - 32
        ghi = y0 + HALF + 32
        clo, chi = max(0, glo), min(H, ghi)
        xpad[:, clo - glo:chi - glo, :] = sharp_image[b, :, clo:chi, :]
        xin = np.zeros((C, NT, 128, W), dtype=np.float32)
        for t in range(NT):
            xin[:, t] = xpad[:, YT * t:YT * t + 128, :]

        if b not in p_full:
            p_full[b] = _plane_index(coc_map[b, 0])
        p = p_full[b][y0:y0 + HALF, :]  # [HALF, W]
        # pmap[m, s, y] = p[y, 128s + m]
        pmap = np.ascontiguousarray(
            p.T.reshape(NS, 128, HALF).transpose(1, 0, 2)).astype(BF16)

        in_maps.append({
            "xin": xin.astype(BF16),
            "t1": t1, "t2m": t2m, "t2l": t2l, "t2r": t2r,
            "pmap": pmap, "ident": ident,
        })
    return in_maps


def _assemble(results):
    out = np.zeros((B, C, H, W), dtype=np.float32)
    for core in range(8):
        b, h = divmod(core, 2)
        r = results[core]["out"]  # [C, 2, 128, 512]
        out[b, :, HALF * h:HALF * (h + 1), :] = r.reshape(C, HALF, W)
    return out


def run(inputs, trace=False):
    from concourse import bass_utils
    if "tables" not in _CACHE:
        _CACHE["tables"] = _host_tables()
    if "nc" not in _CACHE:
        _CACHE["nc"] = _build_program()
    nc = _CACHE["nc"]
    in_maps = _prepare_in_maps(inputs["sharp_image"], inputs["coc_map"])
    res = bass_utils.run_bass_kernel_spmd(
        nc, in_maps, core_ids=list(range(8)), trace=trace)
    return _assemble(res.results), res


def kernel(**inputs):
    out, _ = run(inputs)
    return out
